# revision 33
# baseline (speedup 1.0000x reference)
import numpy as np

VOCAB, EMBED, HIDDEN = 32000, 100, 128
B, T = 1024, 256
NCORES = 8
BS = B // NCORES            # 128 batch rows per core
CH = 2                      # independent scan chains per core
CB = BS // CH               # 64 batch per chain
GCH = 16                    # timesteps per input chunk
NGC = T // GCH              # 16 chunks
LA = 4                      # scan software-pipeline lookahead
NHT = VOCAB // 128          # 250 head tiles of 128 vocab rows

_NC_CACHE = {}


def _build_program():
    from contextlib import ExitStack

    import concourse.mybir as mybir
    import concourse.tile as tile
    from concourse.bacc import Bacc

    f32 = mybir.dt.float32
    Tanh = mybir.ActivationFunctionType.Tanh
    Ident = mybir.ActivationFunctionType.Identity

    nc = Bacc(None, target_bir_lowering=True)
    xg_d = nc.declare_dram_parameter("xg", [128, T, 128], f32, isOutput=False)
    wih_d = nc.declare_dram_parameter("wihT", [128, 128], f32, isOutput=False)
    whh_d = nc.declare_dram_parameter("whhT", [128, 128], f32, isOutput=False)
    wfc_d = nc.declare_dram_parameter("wfcT", [128, VOCAB], f32, isOutput=False)
    bfc_d = nc.declare_dram_parameter("bfc", [128, NHT], f32, isOutput=False)
    out_d = nc.declare_dram_parameter("out", [VOCAB, BS], f32, isOutput=True)

    with tile.TileContext(nc) as tc, ExitStack() as ctx:
        singles = ctx.enter_context(tc.tile_pool(name="singles", bufs=1))
        gpool = ctx.enter_context(tc.tile_pool(name="g", bufs=3))
        zpool = ctx.enter_context(tc.tile_pool(name="z", bufs=6, space="PSUM"))
        hpool = ctx.enter_context(tc.tile_pool(name="h", bufs=4))
        opool = ctx.enter_context(tc.tile_pool(name="o", bufs=2, space="PSUM"))
        spool = ctx.enter_context(tc.tile_pool(name="s", bufs=4))

        wih_sb = singles.tile([128, 128], f32)
        nc.sync.dma_start(wih_sb[:], wih_d[:])
        whh_sb = singles.tile([128, 128], f32)
        nc.sync.dma_start(whh_sb[:], whh_d[:])
        bfc_sb = singles.tile([128, NHT], f32)
        nc.sync.dma_start(bfc_sb[:], bfc_d[:])
        hlast = singles.tile([128, BS], f32)
        wfc_sb = singles.tile([128, VOCAB], f32)
        nc.sync.dma_start(wfc_sb[:], wfc_d[:])

        g_tiles = {}

        def emit_fetch(c):
            g = gpool.tile([128, GCH, 128], f32)
            nc.sync.dma_start(g[:], xg_d[:, c * GCH : (c + 1) * GCH, :])
            g_tiles[c] = g

        for c in range(min(3, NGC)):
            emit_fetch(c)

        h_cur = [None]
        zmap = {}

        def front(t):
            g = g_tiles[t // GCH]
            z = zpool.tile([128, BS], f32, space="PSUM")
            nc.tensor.matmul(
                z[:],
                lhsT=wih_sb[:],
                rhs=g[:, t % GCH, :],
                start=True,
                stop=(t == 0),
            )
            zmap[t] = z
            if t % GCH == GCH - 1 and t // GCH + 3 < NGC:
                emit_fetch(t // GCH + 3)

        def back(t):
            z = zmap.pop(t)
            if t > 0:
                nc.tensor.matmul(
                    z[:], lhsT=whh_sb[:], rhs=h_cur[0][:], start=False, stop=True
                )
            if t < T - 1:
                hn = hpool.tile([128, BS], f32)
                nc.scalar.activation(hn[:], z[:], Tanh)
                h_cur[0] = hn
            else:
                nc.scalar.activation(hlast[:], z[:], Tanh)

        for t in range(T):
            front(t)
            if t >= LA:
                back(t - LA)
        for t in range(T - LA, T):
            back(t)

        for j in range(NHT):
            o = opool.tile([128, BS], f32, space="PSUM")
            nc.tensor.matmul(
                o[:],
                lhsT=wfc_sb[:, j * 128 : (j + 1) * 128],
                rhs=hlast[:],
                start=True,
                stop=True,
            )
            s = spool.tile([128, BS], f32)
            if j % 2 == 0:
                nc.scalar.activation(s[:], o[:], Ident, bias=bfc_sb[:, j : j + 1])
            else:
                nc.vector.tensor_scalar_add(s[:], o[:], bfc_sb[:, j : j + 1])
            nc.sync.dma_start(out_d[j * 128 : (j + 1) * 128, :], s[:])

    return nc


def get_nc():
    if "nc" not in _NC_CACHE:
        nc = _build_program()
        nc.finalize()
        _NC_CACHE["nc"] = nc
    return _NC_CACHE["nc"]


def make_in_maps(x, emb, W_ih, W_hh, b_ih, b_hh, W_fc, b_fc):
    emb_ext = np.zeros((VOCAB, 128), np.float32)
    emb_ext[:, :EMBED] = emb
    emb_ext[:, EMBED] = 1.0
    emb_ext[:, EMBED + 1] = 1.0

    wihT = np.zeros((128, 128), np.float32)
    wihT[:EMBED] = W_ih.T
    wihT[EMBED] = b_ih
    wihT[EMBED + 1] = b_hh

    whhT = np.ascontiguousarray(W_hh.T, np.float32)
    wfcT = np.ascontiguousarray(W_fc.T, np.float32)
    bfc = np.ascontiguousarray(b_fc.reshape(NHT, 128).T, np.float32)

    in_maps = []
    for core in range(NCORES):
        xs = np.asarray(x[core * BS : (core + 1) * BS])  # [128, 256]
        xg = emb_ext[xs]                                 # [128, 256, 128] (b, t, f)
        xg = np.ascontiguousarray(np.transpose(xg, (2, 1, 0)))  # [f, t, b]
        in_maps.append(
            {
                "xg": xg,
                "wihT": wihT,
                "whhT": whhT,
                "wfcT": wfcT,
                "bfc": bfc,
            }
        )
    return in_maps


def kernel(x, emb, W_ih, W_hh, b_ih, b_hh, W_fc, b_fc):
    from concourse.bass_utils import run_bass_kernel_spmd

    nc = get_nc()
    in_maps = make_in_maps(x, emb, W_ih, W_hh, b_ih, b_hh, W_fc, b_fc)
    res = run_bass_kernel_spmd(nc, in_maps, list(range(NCORES)))
    return np.concatenate(
        [np.asarray(res.results[i]["out"]).T for i in range(NCORES)], axis=0
    )


# revision 36
# speedup vs baseline: 1.2054x; 1.2054x over previous
import numpy as np

VOCAB, EMBED, HIDDEN = 32000, 100, 128
B, T = 1024, 256
NCORES = 8
BS = B // NCORES            # 128 batch rows per core
GCH = 16                    # timesteps per input chunk
NGC = T // GCH              # 16 chunks
LA = 4                      # scan software-pipeline lookahead
NHT = VOCAB // 128          # 250 head tiles of 128 vocab rows

_NC_CACHE = {}


def _build_program():
    from contextlib import ExitStack

    import concourse.mybir as mybir
    import concourse.tile as tile
    from concourse.bacc import Bacc

    f32 = mybir.dt.float32
    bf16 = mybir.dt.bfloat16
    Tanh = mybir.ActivationFunctionType.Tanh
    Ident = mybir.ActivationFunctionType.Identity

    nc = Bacc(None, target_bir_lowering=True)
    xg_d = nc.declare_dram_parameter("xg", [128, T, 128], bf16, isOutput=False)
    wih_d = nc.declare_dram_parameter("wihT", [128, 128], bf16, isOutput=False)
    whh_d = nc.declare_dram_parameter("whhT", [128, 128], bf16, isOutput=False)
    wfc_d = nc.declare_dram_parameter("wfcT", [128, VOCAB], bf16, isOutput=False)
    bfc_d = nc.declare_dram_parameter("bfc", [128, NHT], f32, isOutput=False)
    out_d = nc.declare_dram_parameter("out", [VOCAB, BS], bf16, isOutput=True)

    with tile.TileContext(nc) as tc, ExitStack() as ctx:
        singles = ctx.enter_context(tc.tile_pool(name="singles", bufs=1))
        gpool = ctx.enter_context(tc.tile_pool(name="g", bufs=3))
        zpool = ctx.enter_context(tc.tile_pool(name="z", bufs=6, space="PSUM"))
        hpool = ctx.enter_context(tc.tile_pool(name="h", bufs=4))
        opool = ctx.enter_context(tc.tile_pool(name="o", bufs=2, space="PSUM"))
        spool = ctx.enter_context(tc.tile_pool(name="s", bufs=4))

        wih_sb = singles.tile([128, 128], bf16)
        nc.sync.dma_start(wih_sb[:], wih_d[:])
        whh_sb = singles.tile([128, 128], bf16)
        nc.sync.dma_start(whh_sb[:], whh_d[:])
        bfc_sb = singles.tile([128, NHT], f32)
        nc.sync.dma_start(bfc_sb[:], bfc_d[:])
        hlast = singles.tile([128, BS], bf16)
        wfc_sb = singles.tile([128, VOCAB], bf16)
        nc.sync.dma_start(wfc_sb[:], wfc_d[:])

        g_tiles = {}

        def emit_fetch(c):
            g = gpool.tile([128, GCH, 128], bf16)
            nc.sync.dma_start(g[:], xg_d[:, c * GCH : (c + 1) * GCH, :])
            g_tiles[c] = g

        for c in range(min(3, NGC)):
            emit_fetch(c)

        h_cur = [None]
        zmap = {}

        def front(t):
            g = g_tiles[t // GCH]
            z = zpool.tile([128, BS], f32, space="PSUM")
            nc.tensor.matmul(
                z[:],
                lhsT=wih_sb[:],
                rhs=g[:, t % GCH, :],
                start=True,
                stop=(t == 0),
            )
            zmap[t] = z
            if t % GCH == GCH - 1 and t // GCH + 3 < NGC:
                emit_fetch(t // GCH + 3)

        def back(t):
            z = zmap.pop(t)
            if t > 0:
                nc.tensor.matmul(
                    z[:], lhsT=whh_sb[:], rhs=h_cur[0][:], start=False, stop=True
                )
            if t < T - 1:
                hn = hpool.tile([128, BS], bf16)
                nc.scalar.activation(hn[:], z[:], Tanh)
                h_cur[0] = hn
            else:
                nc.scalar.activation(hlast[:], z[:], Tanh)

        for t in range(T):
            front(t)
            if t >= LA:
                back(t - LA)
        for t in range(T - LA, T):
            back(t)

        for j in range(NHT):
            o = opool.tile([128, BS], f32, space="PSUM")
            nc.tensor.matmul(
                o[:],
                lhsT=wfc_sb[:, j * 128 : (j + 1) * 128],
                rhs=hlast[:],
                start=True,
                stop=True,
            )
            s = spool.tile([128, BS], bf16)
            if j % 2 == 0:
                nc.scalar.activation(s[:], o[:], Ident, bias=bfc_sb[:, j : j + 1])
            else:
                nc.vector.tensor_scalar_add(s[:], o[:], bfc_sb[:, j : j + 1])
            nc.sync.dma_start(out_d[j * 128 : (j + 1) * 128, :], s[:])

    return nc


def get_nc():
    if "nc" not in _NC_CACHE:
        nc = _build_program()
        nc.finalize()
        _NC_CACHE["nc"] = nc
    return _NC_CACHE["nc"]


def make_in_maps(x, emb, W_ih, W_hh, b_ih, b_hh, W_fc, b_fc):
    import ml_dtypes

    bf16 = ml_dtypes.bfloat16

    emb_ext = np.zeros((VOCAB, 128), np.float32)
    emb_ext[:, :EMBED] = emb
    emb_ext[:, EMBED] = 1.0
    emb_ext[:, EMBED + 1] = 1.0

    wihT = np.zeros((128, 128), np.float32)
    wihT[:EMBED] = W_ih.T
    wihT[EMBED] = b_ih
    wihT[EMBED + 1] = b_hh
    wihT = wihT.astype(bf16)

    whhT = np.ascontiguousarray(W_hh.T).astype(bf16)
    wfcT = np.ascontiguousarray(W_fc.T).astype(bf16)
    bfc = np.ascontiguousarray(b_fc.reshape(NHT, 128).T, np.float32)

    in_maps = []
    for core in range(NCORES):
        xs = np.asarray(x[core * BS : (core + 1) * BS])  # [128, 256]
        xg = emb_ext[xs]                                 # [128, 256, 128] (b, t, f)
        xg = np.ascontiguousarray(np.transpose(xg, (2, 1, 0))).astype(bf16)
        in_maps.append(
            {
                "xg": xg,
                "wihT": wihT,
                "whhT": whhT,
                "wfcT": wfcT,
                "bfc": bfc,
            }
        )
    return in_maps


def kernel(x, emb, W_ih, W_hh, b_ih, b_hh, W_fc, b_fc):
    from concourse.bass_utils import run_bass_kernel_spmd

    nc = get_nc()
    in_maps = make_in_maps(x, emb, W_ih, W_hh, b_ih, b_hh, W_fc, b_fc)
    res = run_bass_kernel_spmd(nc, in_maps, list(range(NCORES)))
    return np.concatenate(
        [np.asarray(res.results[i]["out"]).astype(np.float32).T for i in range(NCORES)],
        axis=0,
    )


# revision 42
# speedup vs baseline: 1.8958x; 1.5727x over previous
import numpy as np

VOCAB, EMBED, HIDDEN = 32000, 100, 128
B, T = 1024, 256
NCORES = 8
BS = B // NCORES            # 128 batch rows per core
GCH = 16                    # timesteps per input chunk
NGC = T // GCH              # 16 chunks
LA = 4                      # scan software-pipeline lookahead
HT = 512                    # head tile width (one PSUM bank of f32)
WP = VOCAB // NGC           # wfc columns streamed per scan chunk

_NC_CACHE = {}


def _build_program():
    from contextlib import ExitStack

    import concourse.mybir as mybir
    import concourse.tile as tile
    from concourse.bacc import Bacc

    f32 = mybir.dt.float32
    bf16 = mybir.dt.bfloat16
    Tanh = mybir.ActivationFunctionType.Tanh
    Ident = mybir.ActivationFunctionType.Identity

    nc = Bacc(None, target_bir_lowering=True)
    xg_d = nc.declare_dram_parameter("xg", [128, T, 128], bf16, isOutput=False)
    wih_d = nc.declare_dram_parameter("wihT", [128, 128], bf16, isOutput=False)
    whh_d = nc.declare_dram_parameter("whhT", [128, 128], bf16, isOutput=False)
    wfc_d = nc.declare_dram_parameter("wfcT", [128, VOCAB], bf16, isOutput=False)
    out_d = nc.declare_dram_parameter("out", [BS, VOCAB], bf16, isOutput=True)

    with tile.TileContext(nc) as tc, ExitStack() as ctx:
        singles = ctx.enter_context(tc.tile_pool(name="singles", bufs=1))
        gpool = ctx.enter_context(tc.tile_pool(name="g", bufs=3))
        zpool = ctx.enter_context(tc.tile_pool(name="z", bufs=6, space="PSUM"))
        hpool = ctx.enter_context(tc.tile_pool(name="h", bufs=4))
        opool = ctx.enter_context(tc.tile_pool(name="o", bufs=2, space="PSUM"))
        spool = ctx.enter_context(tc.tile_pool(name="s", bufs=4))

        wih_sb = singles.tile([128, 128], bf16)
        nc.sync.dma_start(wih_sb[:], wih_d[:])
        whh_sb = singles.tile([128, 128], bf16)
        nc.sync.dma_start(whh_sb[:], whh_d[:])
        hlast = singles.tile([128, BS], bf16)
        wfc_sb = singles.tile([128, VOCAB], bf16)

        g_tiles = {}

        def emit_fetch(c):
            g = gpool.tile([128, GCH, 128], bf16)
            nc.sync.dma_start(g[:], xg_d[:, c * GCH : (c + 1) * GCH, :])
            nc.sync.dma_start(
                wfc_sb[:, c * WP : (c + 1) * WP], wfc_d[:, c * WP : (c + 1) * WP]
            )
            g_tiles[c] = g

        for c in range(min(3, NGC)):
            emit_fetch(c)

        h_cur = [None]
        zmap = {}

        def front(t):
            g = g_tiles[t // GCH]
            z = zpool.tile([128, BS], f32, space="PSUM")
            nc.tensor.matmul(
                z[:],
                lhsT=wih_sb[:],
                rhs=g[:, t % GCH, :],
                start=True,
                stop=(t == 0),
            )
            zmap[t] = z
            if t % GCH == GCH - 1 and t // GCH + 3 < NGC:
                emit_fetch(t // GCH + 3)

        def back(t):
            z = zmap.pop(t)
            if t > 0:
                nc.tensor.matmul(
                    z[:], lhsT=whh_sb[:], rhs=h_cur[0][:], start=False, stop=True
                )
            if t < T - 1:
                hn = hpool.tile([128, BS], bf16)
                nc.scalar.activation(hn[:], z[:], Tanh)
                h_cur[0] = hn
            else:
                nc.scalar.activation(hlast[:], z[:], Tanh)

        for t in range(T):
            front(t)
            if t >= LA:
                back(t - LA)
        for t in range(T - LA, T):
            back(t)

        off = 0
        j = 0
        while off < VOCAB:
            w = min(HT, VOCAB - off)
            o = opool.tile([128, w], f32, space="PSUM")
            nc.tensor.matmul(
                o[:], lhsT=hlast[:], rhs=wfc_sb[:, off : off + w], start=True, stop=True
            )
            s = spool.tile([128, w], bf16)
            if j % 2 == 0:
                nc.scalar.activation(s[:], o[:], Ident)
            else:
                nc.vector.tensor_copy(s[:], o[:])
            nc.sync.dma_start(out_d[:, off : off + w], s[:])
            off += w
            j += 1

    return nc


def get_nc():
    if "nc" not in _NC_CACHE:
        nc = _build_program()
        nc.finalize()
        _NC_CACHE["nc"] = nc
    return _NC_CACHE["nc"]


def make_in_maps(x, emb, W_ih, W_hh, b_ih, b_hh, W_fc, b_fc):
    import ml_dtypes

    bf16 = ml_dtypes.bfloat16

    emb_ext = np.zeros((VOCAB, 128), np.float32)
    emb_ext[:, :EMBED] = emb
    emb_ext[:, EMBED] = 1.0
    emb_ext[:, EMBED + 1] = 1.0

    wihT = np.zeros((128, 128), np.float32)
    wihT[:EMBED] = W_ih.T
    wihT[EMBED] = b_ih
    wihT[EMBED + 1] = b_hh
    wihT = wihT.astype(bf16)

    whhT = np.ascontiguousarray(W_hh.T).astype(bf16)
    wfcT = np.ascontiguousarray(W_fc.T).astype(bf16)

    in_maps = []
    for core in range(NCORES):
        xs = np.asarray(x[core * BS : (core + 1) * BS])  # [128, 256]
        xg = emb_ext[xs]                                 # [128, 256, 128] (b, t, f)
        xg = np.ascontiguousarray(np.transpose(xg, (2, 1, 0))).astype(bf16)
        in_maps.append(
            {
                "xg": xg,
                "wihT": wihT,
                "whhT": whhT,
                "wfcT": wfcT,
            }
        )
    return in_maps


def kernel(x, emb, W_ih, W_hh, b_ih, b_hh, W_fc, b_fc):
    from concourse.bass_utils import run_bass_kernel_spmd

    nc = get_nc()
    in_maps = make_in_maps(x, emb, W_ih, W_hh, b_ih, b_hh, W_fc, b_fc)
    res = run_bass_kernel_spmd(nc, in_maps, list(range(NCORES)))
    out = np.concatenate(
        [np.asarray(res.results[i]["out"]).astype(np.float32) for i in range(NCORES)],
        axis=0,
    )
    out += np.asarray(b_fc, np.float32)[None, :]
    return out
